# revision 18
# baseline (speedup 1.0000x reference)
"""Trainium2 Bass kernel for a Bayesian (variational) single-layer LSTM.

Reference computation (B=512, S=128, H=512, IN=1, OUT=1):
    W = mu + softplus(rho) * eps            (variational reparameterization)
    u[b,s] = x[b,s] * mask_in[b,s]          (inverted input dropout)
    gates(t) = u[:,t] * W_ih + b + h(t-1) @ W_hh
    i,f,g,o = split(gates); c = f*c + i*g; h = o * tanh(c)
    out = (h(S-1) * mask_out) @ W_lin + b_lin

Strategy: data-parallel over batch (64 rows per core, 8 cores), weights
replicated.  The variational sampling / scaling / transposes are pure
input preprocessing and run on the host (numpy), so the device prologue
is just ~2 MB of fp16 weight DMA.  Everything on-chip is kept in a
feature-major ("transposed") layout: gates^T / h^T / c^T with gate
features on partitions and batch on the free dim, so the recurrence
needs no transposes:

    gates^T[128-feat tile m, batch] = sum_k W_hh[k-chunk, m-tile].T @ h^T[k-chunk]
                                      (+ [W_ih; b].T @ [u_s; 1])

Per step: 64 k-matmuls (N=64, issue at ~29 ns warm) ordered gate-major
g,i,f,o so each gate's PSUM bank completes just before its sigmoid; the
ACT pipeline sigma_g -> sigma_i -> sigma_f -> sigma_o overlaps the
remaining matmuls.  The elementwise chain is sigmoid-only
(tanh(x) = 2*sig(2x)-1 pre-folded into weight scaling; cell stored as
C = c/2, h as h/2 with W_lin doubled).  The fp32 DVE chain
t=(sg-.5)*si, fc=sf*C, C'=t+fc, then sig(4C') and h=(sc-.5)*so.

HAM: the PE clock gate re-throttles to 1.2 GHz on idle windows, which
doubles every matmul.  Scratch-bank filler matmuls -- dep-gated on this
step's sigmoids so they spread across the chain tail -- plus warm-up
matmuls during the prologue DMA keep the PE at 2.4 GHz for the whole
kernel (measured: k4 residency drops from 44% to ~1%).

Precision: W/h/u fp16, PSUM + sigmoids + DVE chain fp32.  fp16 in the
sigma/t/c path measures 2.1-2.5e-2 end-to-end on HW (amplified through
the recurrence) -- over the 2e-2 budget -- so the chain stays fp32.
Rel err vs the fp32 reference: 9.3e-3.
"""

import os
import sys

import numpy as np

for _p in ("/opt/trn_rl_repo",):
    if _p not in sys.path:
        sys.path.insert(0, _p)

from concourse import bacc, bass, mybir, tile  # noqa: E402
from concourse.bass_utils import run_bass_kernel_spmd  # noqa: E402
from concourse.tile_rust import add_dep_helper  # noqa: E402

N_CORES = 8
B, S, H, OUT = 512, 128, 512, 1
BL = B // N_CORES            # 64 batch rows per core
G4 = 4 * H                   # 2048 gate features
KC = H // 128                # 4 contraction chunks
MT = G4 // 128               # 16 gate-feature tiles
F32 = mybir.dt.float32
F16 = mybir.dt.float16
AF = mybir.ActivationFunctionType
OP = mybir.AluOpType

_cache = {}


def _build():
    if "built" in _cache:
        return _cache["built"]

    nc = bacc.Bacc(
        "TRN2", target_bir_lowering=False, debug=False, num_devices=N_CORES
    )

    # ---- I/O (all preprocessed on host) ----
    u2_in = nc.dram_tensor("u2_in", [2, S * BL], F16, kind="ExternalInput").ap()
    wg_in = nc.dram_tensor("wg_in", [2, G4], F16, kind="ExternalInput").ap()
    w16_in = nc.dram_tensor(
        "w16_in", [128, KC * G4], F16, kind="ExternalInput"
    ).ap()
    mw_in = nc.dram_tensor(
        "mw_in", [128, KC * BL + KC], F16, kind="ExternalInput"
    ).ap()
    bl_in = nc.dram_tensor("bl_in", [1, OUT], F32, kind="ExternalInput").ap()
    out_d = nc.dram_tensor("out", [BL, OUT], F32, kind="ExternalOutput").ap()

    with tile.TileContext(nc) as tc:
        with tc.tile_pool(name="const", bufs=1) as const:
            u2 = const.tile([2, S * BL], F16, tag="u2", name="u2")
            wg = const.tile([2, G4], F16, tag="wg", name="wg")
            w16cat = const.tile([128, KC * G4], F16, tag="w16", name="w16")
            w16 = [w16cat[:, G4 * k:G4 * (k + 1)] for k in range(KC)]
            mw = const.tile([128, KC * BL + KC], F16, tag="mw", name="mw")
            mot = mw[:, 0:KC * BL]
            wl16 = mw[:, KC * BL:KC * BL + KC]
            bl32 = const.tile([1, OUT], F32, tag="bl32", name="bl32")
            # warm-up operands: no DMA dependency, so the PE can start
            # immediately at kernel entry and unthrottle the HAM clock
            # gate while the weight DMAs stream in.
            warm = const.tile([128, 640], F16, tag="warm", name="warm")

            nc.gpsimd.memset(warm[:, 0:2], 0.25)
            nc.sync.dma_start(u2[:, :], u2_in)
            nc.sync.dma_start(wg[:, :], wg_in)
            half = KC * G4 // 2
            nc.sync.dma_start(w16cat[:, 0:half], w16_in[:, 0:half])
            nc.sync.dma_start(w16cat[:, half:], w16_in[:, half:])
            nc.sync.dma_start(mw[:, :], mw_in)
            nc.sync.dma_start(bl32[:, :], bl_in)

            # ------------- recurrence -------------
            # One PSUM bank per gate (bufs=1: banks reused in place; the
            # WAR dep this creates -- next step's gx opener waits for this
            # step's sigma on the same bank -- spreads the gx matmuls
            # across the chain tail, which doubles as HAM keep-alive).
            with tc.tile_pool(name="work", bufs=4) as work:
              with tc.tile_pool(name="psum", bufs=1, space="PSUM") as psum:
                h_prev = None
                c_prev = None
                # gate order: g first (tanh arg, needed first), then i, f, o
                GATES = (2, 0, 1, 3)   # m-tile group: gate*4 .. gate*4+3

                def make_banks(s):
                    return {
                        gt: psum.tile(
                            [128, 256], F32, tag=f"ps{gt}", name=f"ps{gt}_{s}"
                        )
                        for gt in GATES
                    }

                def gx_into(pbk, s):
                    u_s = u2[:, BL * s:BL * (s + 1)]
                    for gt in GATES:
                        opener = None
                        for m in range(4 * gt, 4 * gt + 4):
                            col = 64 * (m % 4)
                            r = nc.tensor.matmul(
                                pbk[gt][:, col:col + 64],
                                wg[:, 128 * m:128 * (m + 1)],
                                u_s,
                                start=(opener is None), stop=False,
                                skip_group_check=True,
                            )
                            if opener is None:
                                opener = r
                            else:
                                add_dep_helper(
                                    r.ins, opener.ins, reason="bank start first"
                                )

                # ~4.3us of back-to-back warm-up matmuls at kernel entry:
                # one HAM SHORT window of sustained PE activity flips the
                # clock gate to 2.4 GHz before step 0's real matmuls.
                for wi in range(10):
                    scr = psum.tile(
                        [128, 512], F32, tag="scr0", name=f"warm_{wi}"
                    )
                    nc.tensor.matmul(
                        scr[:, :], warm[:, 0:128], warm[:, 128:640],
                        start=True, stop=True, skip_group_check=True,
                    )
                pbk = make_banks(0)
                gx_into(pbk, 0)
                for s in range(S):
                    if h_prev is not None:
                        # gate-major, k-minor: gate g's bank is complete
                        # after its 16 matmuls so sigma_g starts 3 gates
                        # early; k0/k1 read the first h half (ready one
                        # DVE op before the second).
                        for gt in GATES:
                            for k in range(KC):
                                for m in range(4 * gt, 4 * gt + 4):
                                    col = 64 * (m % 4)
                                    nc.tensor.matmul(
                                        pbk[gt][:, col:col + 64],
                                        w16[k][:, 128 * m:128 * (m + 1)],
                                        h_prev[:, 64 * k:64 * (k + 1)],
                                        start=False,
                                        stop=(k == KC - 1 and m == 4 * gt + 3),
                                        skip_group_check=True,
                                    )
                    # elementwise, sigmoid-only (tanh(x) = 2*sig(2x)-1 with
                    # the doubling pre-folded into the weights).  Cell state
                    # C = c/2:  C = (sig(2g)-.5)*i_t + f_t*C_prev,
                    # h/2 = (sig(4C)-.5)*o_t.  One sigma per gate bank.
                    sg = psum.tile([128, 256], F32, tag="sgp", name=f"sg_{s}")
                    r_sg = nc.scalar.activation(sg[:, :], pbk[2][:, :], AF.Sigmoid)
                    si = work.tile([128, 256], F32, tag="si", name=f"si_{s}")
                    r_si = nc.scalar.activation(si[:, :], pbk[0][:, :], AF.Sigmoid)
                    sf = work.tile([128, 256], F32, tag="sf", name=f"sf_{s}")
                    r_sf = nc.scalar.activation(sf[:, :], pbk[1][:, :], AF.Sigmoid)
                    so = work.tile([128, 256], F32, tag="so", name=f"so_{s}")
                    r_so = nc.scalar.activation(so[:, :], pbk[3][:, :], AF.Sigmoid)

                    # HAM keep-warm fillers, dep-gated on this step's sigmas
                    # so they execute spread across the chain tail.
                    for fi, r_sig in enumerate(
                        (r_sg, r_sf, r_so) if s < S - 3 else ()
                    ):
                        scr = psum.tile(
                            [128, 512], F32, tag="scr0", name=f"scr_{s}_{fi}"
                        )
                        r_fill = nc.tensor.matmul(
                            scr[:, :], w16[0][:, 0:128], w16[1][:, 0:512],
                            start=True, stop=True, skip_group_check=True,
                        )
                        add_dep_helper(
                            r_fill.ins, r_sig.ins, reason="spread fillers"
                        )

                    if s + 1 < S:
                        nxt = make_banks(s + 1)
                        gx_into(nxt, s + 1)

                    t = work.tile([128, 256], F32, tag="tg", name=f"tg_{s}")
                    nc.vector.scalar_tensor_tensor(
                        t[:, :], sg[:, :], 0.5, si[:, :],
                        op0=OP.subtract, op1=OP.mult,
                    )
                    if c_prev is None:
                        c_new = t
                    else:
                        fc = work.tile([128, 256], F32, tag="fc", name=f"fc_{s}")
                        nc.vector.tensor_mul(fc[:, :], sf[:, :], c_prev[:, :])
                        c_new = psum.tile([128, 256], F32, tag="cTp", name=f"cT_{s}")
                        nc.vector.tensor_add(c_new[:, :], t[:, :], fc[:, :])
                    h_new = work.tile([128, 256], F16, tag="hT", name=f"hT_{s}")
                    for hf in range(2):
                        sl = slice(128 * hf, 128 * (hf + 1))
                        # sc0 via PSUM: ScE->PSUM write is the faster port
                        # and sigma_c0 is on the critical path.
                        sc = (psum if hf == 0 else work).tile(
                            [128, 128], F32, tag=f"sc{hf}", name=f"sc{hf}_{s}"
                        )
                        nc.scalar.activation(
                            sc[:, :], c_new[:, sl], AF.Sigmoid, scale=4.0
                        )
                        nc.vector.scalar_tensor_tensor(
                            h_new[:, sl], sc[:, :], 0.5, so[:, sl],
                            op0=OP.subtract, op1=OP.mult,
                        )
                    h_prev, c_prev = h_new, c_new
                    if s + 1 < S:
                        pbk = nxt

              # ------------- epilogue -------------
              with tc.tile_pool(name="psum2", bufs=1, space="PSUM") as psum2:
                mh = work.tile([128, KC * BL], F16, tag="mh", name="mh")
                nc.vector.tensor_mul(mh[:, :], h_prev[:, :], mot[:, :])
                pso = psum2.tile([1, BL], F32, tag="pso", name="pso", bufs=1)
                for k in range(KC):
                    nc.tensor.matmul(
                        pso[0:1, :],
                        wl16[:, k:k + 1],
                        mh[:, BL * k:BL * (k + 1)],
                        start=(k == 0), stop=(k == KC - 1),
                    )
                osb = work.tile([1, BL], F32, tag="osb", name="osb")
                nc.vector.tensor_scalar(
                    osb[:, :], pso[0:1, :], bl32[0:1, 0:1], None, op0=OP.add
                )
                nc.sync.dma_start(out_d.rearrange("b o -> o b"), osb[:, :])

    nc.compile()
    _cache["built"] = nc
    return nc


def kernel(**inputs) -> np.ndarray:
    nc = _build()
    f32 = np.float32
    f16 = np.float16

    def c(a):
        return np.asarray(a, dtype=f32)

    # ---- host-side preprocessing (variational sampling + layout) ----
    sp_ih = np.logaddexp(0.0, c(inputs["W_ih_rho"]))       # softplus
    W_ih = (c(inputs["W_ih_mu"]) + sp_ih * c(inputs["eps_ih"])).reshape(G4)
    sp_b = np.logaddexp(0.0, c(inputs["b_rho"]))
    bias = (c(inputs["b_mu"]) + sp_b * c(inputs["eps_b"])).reshape(G4)
    sp_hh = np.logaddexp(0.0, c(inputs["W_hh_rho"]))
    W_hh = c(inputs["W_hh_mu"]) + sp_hh * c(inputs["eps_hh"])   # (H, 4H)

    # scaling tricks: tanh(x) = 2*sig(2x)-1 needs the g-gate (cols
    # 1024:1536) pre-activations doubled; storing h/2 needs all W_hh
    # columns doubled (and W_lin doubled at the output).
    sc_row = np.ones(G4, f32)
    sc_row[1024:1536] = 2.0
    sc_w = np.full(G4, 2.0, f32)
    sc_w[1024:1536] = 4.0
    wg_np = np.ascontiguousarray(
        np.stack([W_ih * sc_row, bias * sc_row]).astype(f16)
    )
    whh16 = (W_hh * sc_w).astype(f16)                       # (512, 2048)
    w16_np = [
        np.ascontiguousarray(whh16[128 * k:128 * (k + 1)]) for k in range(KC)
    ]
    wl2 = (2.0 * c(inputs["W_lin"]).reshape(KC, 128).T).astype(f16)
    wl16_np = np.ascontiguousarray(wl2)                     # (128, KC)
    bl_np = c(inputs["b_lin"]).reshape(1, OUT)

    u_full = (c(inputs["x"]) * c(inputs["mask_in"]).reshape(B, S)).astype(f16)
    mo_full = c(inputs["mask_out"]).astype(f16)             # (B, H)

    w16cat_np = np.ascontiguousarray(np.concatenate(w16_np, axis=1))
    shared = {"wg_in": wg_np, "w16_in": w16cat_np, "bl_in": bl_np}
    in_maps = []
    for i in range(N_CORES):
        sl = slice(BL * i, BL * (i + 1))
        m = dict(shared)
        u_sl = u_full[sl]                                   # (BL, S)
        m["u2_in"] = np.ascontiguousarray(
            np.stack([u_sl.T.ravel(), np.ones(S * BL, f16)])
        )
        mo_sl = mo_full[sl]                                 # (BL, H)
        m["mw_in"] = np.ascontiguousarray(
            np.concatenate(
                [mo_sl[:, 128 * k:128 * (k + 1)].T for k in range(KC)]
                + [wl16_np], axis=1
            )
        )
        in_maps.append(m)

    trace = bool(int(os.environ.get("KERNEL_TRACE", "0")))
    trace_cores = None
    if trace and int(os.environ.get("KERNEL_TRACE_ALL", "0")):
        trace_cores = list(range(N_CORES))
    res = None
    last_err = None
    for _attempt in range(3):
        try:
            res = run_bass_kernel_spmd(
                nc, in_maps, core_ids=list(range(N_CORES)), trace=trace,
                trace_cores=trace_cores,
            )
            break
        except Exception as e:  # transient NRT/device hiccups: retry
            last_err = e
    if res is None:
        raise last_err
    _cache["last_results"] = res
    out = np.concatenate(
        [res.results[i]["out"].reshape(BL, OUT) for i in range(N_CORES)], axis=0
    )
    return out.astype(np.float32)


# revision 19
# speedup vs baseline: 1.1975x; 1.1975x over previous
"""Trainium2 Bass kernel for a Bayesian (variational) single-layer LSTM.

Reference computation (B=512, S=128, H=512, IN=1, OUT=1):
    W = mu + softplus(rho) * eps            (variational reparameterization)
    u[b,s] = x[b,s] * mask_in[b,s]          (inverted input dropout)
    gates(t) = u[:,t] * W_ih + b + h(t-1) @ W_hh
    i,f,g,o = split(gates); c = f*c + i*g; h = o * tanh(c)
    out = (h(S-1) * mask_out) @ W_lin + b_lin

Strategy: data-parallel over batch (64 rows per core, 8 cores), weights
replicated.  The variational sampling / scaling / transposes are pure
input preprocessing and run on the host (numpy), so the device prologue
is just ~2 MB of fp16 weight DMA.  Everything on-chip is kept in a
feature-major ("transposed") layout: gates^T / h^T / c^T with gate
features on partitions and batch on the free dim, so the recurrence
needs no transposes:

    gates^T[128-feat tile m, batch] = sum_k W_hh[k-chunk, m-tile].T @ h^T[k-chunk]
                                      (+ [W_ih; b].T @ [u_s; 1])

Per step: 64 k-matmuls (N=64, issue at ~29 ns warm) ordered gate-major
g,i,f,o so each gate's PSUM bank completes just before its sigmoid; the
ACT pipeline sigma_g -> sigma_i -> sigma_f -> sigma_o overlaps the
remaining matmuls.  The elementwise chain is sigmoid-only
(tanh(x) = 2*sig(2x)-1 pre-folded into weight scaling; cell stored as
C = c/2, h as h/2 with W_lin doubled).  The fp32 DVE chain
t=(sg-.5)*si, fc=sf*C, C'=t+fc, then sig(4C') and h=(sc-.5)*so.

HAM: the PE clock gate re-throttles to 1.2 GHz on idle windows, which
doubles every matmul.  Scratch-bank filler matmuls -- dep-gated on this
step's sigmoids so they spread across the chain tail -- plus warm-up
matmuls during the prologue DMA keep the PE at 2.4 GHz for the whole
kernel (measured: k4 residency drops from 44% to ~1%).

Precision: W/h/u fp16, PSUM + sigmoids + DVE chain fp32.  fp16 in the
sigma/t/c path measures 2.1-2.5e-2 end-to-end on HW (amplified through
the recurrence) -- over the 2e-2 budget -- so the chain stays fp32.
Rel err vs the fp32 reference: 9.3e-3.
"""

import os
import sys

import numpy as np

for _p in ("/opt/trn_rl_repo",):
    if _p not in sys.path:
        sys.path.insert(0, _p)

from concourse import bacc, bass, mybir, tile  # noqa: E402
from concourse.bass_utils import run_bass_kernel_spmd  # noqa: E402
from concourse.tile_rust import add_dep_helper  # noqa: E402

N_CORES = 8
B, S, H, OUT = 512, 128, 512, 1
BL = B // N_CORES            # 64 batch rows per core
G4 = 4 * H                   # 2048 gate features
KC = H // 128                # 4 contraction chunks
MT = G4 // 128               # 16 gate-feature tiles
F32 = mybir.dt.float32
F16 = mybir.dt.float16
AF = mybir.ActivationFunctionType
OP = mybir.AluOpType

_cache = {}


def _build():
    if "built" in _cache:
        return _cache["built"]

    nc = bacc.Bacc(
        "TRN2", target_bir_lowering=False, debug=False, num_devices=N_CORES
    )

    # ---- I/O (all preprocessed on host) ----
    u2_in = nc.dram_tensor("u2_in", [2, S * BL], F16, kind="ExternalInput").ap()
    wg_in = nc.dram_tensor("wg_in", [2, G4], F16, kind="ExternalInput").ap()
    w16_in = nc.dram_tensor(
        "w16_in", [128, KC * G4], F16, kind="ExternalInput"
    ).ap()
    mw_in = nc.dram_tensor(
        "mw_in", [128, KC * BL + KC], F16, kind="ExternalInput"
    ).ap()
    bl_in = nc.dram_tensor("bl_in", [1, OUT], F32, kind="ExternalInput").ap()
    out_d = nc.dram_tensor("out", [BL, OUT], F32, kind="ExternalOutput").ap()

    with tile.TileContext(nc) as tc:
        with tc.tile_pool(name="const", bufs=1) as const:
            u2 = const.tile([2, S * BL], F16, tag="u2", name="u2")
            wg = const.tile([2, G4], F16, tag="wg", name="wg")
            w16cat = const.tile([128, KC * G4], F16, tag="w16", name="w16")
            w16 = [w16cat[:, G4 * k:G4 * (k + 1)] for k in range(KC)]
            mw = const.tile([128, KC * BL + KC], F16, tag="mw", name="mw")
            mot = mw[:, 0:KC * BL]
            wl16 = mw[:, KC * BL:KC * BL + KC]
            bl32 = const.tile([1, OUT], F32, tag="bl32", name="bl32")
            # warm-up operands: no DMA dependency, so the PE can start
            # immediately at kernel entry and unthrottle the HAM clock
            # gate while the weight DMAs stream in.
            warm = const.tile([128, 640], F16, tag="warm", name="warm")

            nc.gpsimd.memset(warm[:, 0:2], 0.25)
            nc.sync.dma_start(u2[:, :], u2_in)
            nc.sync.dma_start(wg[:, :], wg_in)
            nc.sync.dma_start(w16cat[:, :], w16_in)
            nc.sync.dma_start(mw[:, :], mw_in)
            nc.sync.dma_start(bl32[:, :], bl_in)

            # ------------- recurrence -------------
            # One PSUM bank per gate (bufs=1: banks reused in place; the
            # WAR dep this creates -- next step's gx opener waits for this
            # step's sigma on the same bank -- spreads the gx matmuls
            # across the chain tail, which doubles as HAM keep-alive).
            with tc.tile_pool(name="work", bufs=4) as work:
              with tc.tile_pool(name="psum", bufs=1, space="PSUM") as psum:
                h_prev = None
                c_prev = None
                # gate order: g first (tanh arg, needed first), then i, f, o
                GATES = (2, 0, 1, 3)   # m-tile group: gate*4 .. gate*4+3

                def make_banks(s):
                    return {
                        gt: psum.tile(
                            [128, 256], F32, tag=f"ps{gt}", name=f"ps{gt}_{s}"
                        )
                        for gt in GATES
                    }

                def gx_into(pbk, s):
                    u_s = u2[:, BL * s:BL * (s + 1)]
                    for gt in GATES:
                        opener = None
                        for m in range(4 * gt, 4 * gt + 4):
                            col = 64 * (m % 4)
                            r = nc.tensor.matmul(
                                pbk[gt][:, col:col + 64],
                                wg[:, 128 * m:128 * (m + 1)],
                                u_s,
                                start=(opener is None), stop=False,
                                skip_group_check=True,
                            )
                            if opener is None:
                                opener = r
                            else:
                                add_dep_helper(
                                    r.ins, opener.ins, reason="bank start first"
                                )

                # ~4.3us of back-to-back warm-up matmuls at kernel entry:
                # one HAM SHORT window of sustained PE activity flips the
                # clock gate to 2.4 GHz before step 0's real matmuls.
                for wi in range(10):
                    scr = psum.tile(
                        [128, 512], F32, tag="scr0", name=f"warm_{wi}"
                    )
                    nc.tensor.matmul(
                        scr[:, :], warm[:, 0:128], warm[:, 128:640],
                        start=True, stop=True, skip_group_check=True,
                    )
                pbk = make_banks(0)
                gx_into(pbk, 0)
                for s in range(S):
                    if h_prev is not None:
                        # gate-major, k-minor: gate g's bank is complete
                        # after its 16 matmuls so sigma_g starts 3 gates
                        # early; k0/k1 read the first h half (ready one
                        # DVE op before the second).
                        for gt in GATES:
                            for k in range(KC):
                                for m in range(4 * gt, 4 * gt + 4):
                                    col = 64 * (m % 4)
                                    nc.tensor.matmul(
                                        pbk[gt][:, col:col + 64],
                                        w16[k][:, 128 * m:128 * (m + 1)],
                                        h_prev[:, 64 * k:64 * (k + 1)],
                                        start=False,
                                        stop=(k == KC - 1 and m == 4 * gt + 3),
                                        skip_group_check=True,
                                    )
                    # elementwise, sigmoid-only (tanh(x) = 2*sig(2x)-1 with
                    # the doubling pre-folded into the weights).  Cell state
                    # C = c/2:  C = (sig(2g)-.5)*i_t + f_t*C_prev,
                    # h/2 = (sig(4C)-.5)*o_t.  One sigma per gate bank.
                    sg = psum.tile([128, 256], F32, tag="sgp", name=f"sg_{s}")
                    r_sg = nc.scalar.activation(sg[:, :], pbk[2][:, :], AF.Sigmoid)
                    si = work.tile([128, 256], F32, tag="si", name=f"si_{s}")
                    r_si = nc.scalar.activation(si[:, :], pbk[0][:, :], AF.Sigmoid)
                    sf = work.tile([128, 256], F32, tag="sf", name=f"sf_{s}")
                    r_sf = nc.scalar.activation(sf[:, :], pbk[1][:, :], AF.Sigmoid)
                    so = work.tile([128, 256], F32, tag="so", name=f"so_{s}")
                    r_so = nc.scalar.activation(so[:, :], pbk[3][:, :], AF.Sigmoid)

                    # HAM keep-warm fillers, dep-gated on this step's sigmas
                    # so they execute spread across the chain tail.
                    for fi, r_sig in enumerate(
                        (r_sg, r_sf, r_so) if s < S - 3 else ()
                    ):
                        scr = psum.tile(
                            [128, 512], F32, tag="scr0", name=f"scr_{s}_{fi}"
                        )
                        r_fill = nc.tensor.matmul(
                            scr[:, :], w16[0][:, 0:128], w16[1][:, 0:512],
                            start=True, stop=True, skip_group_check=True,
                        )
                        add_dep_helper(
                            r_fill.ins, r_sig.ins, reason="spread fillers"
                        )

                    if s + 1 < S:
                        nxt = make_banks(s + 1)
                        gx_into(nxt, s + 1)

                    t = work.tile([128, 256], F32, tag="tg", name=f"tg_{s}")
                    nc.vector.scalar_tensor_tensor(
                        t[:, :], sg[:, :], 0.5, si[:, :],
                        op0=OP.subtract, op1=OP.mult,
                    )
                    if c_prev is None:
                        c_new = t
                    else:
                        fc = work.tile([128, 256], F32, tag="fc", name=f"fc_{s}")
                        nc.vector.tensor_mul(fc[:, :], sf[:, :], c_prev[:, :])
                        c_new = psum.tile([128, 256], F32, tag="cTp", name=f"cT_{s}")
                        nc.vector.tensor_add(c_new[:, :], t[:, :], fc[:, :])
                    h_new = work.tile([128, 256], F16, tag="hT", name=f"hT_{s}")
                    for hf in range(2):
                        sl = slice(128 * hf, 128 * (hf + 1))
                        # sc0 via PSUM: ScE->PSUM write is the faster port
                        # and sigma_c0 is on the critical path.
                        sc = (psum if hf == 0 else work).tile(
                            [128, 128], F32, tag=f"sc{hf}", name=f"sc{hf}_{s}"
                        )
                        nc.scalar.activation(
                            sc[:, :], c_new[:, sl], AF.Sigmoid, scale=4.0
                        )
                        nc.vector.scalar_tensor_tensor(
                            h_new[:, sl], sc[:, :], 0.5, so[:, sl],
                            op0=OP.subtract, op1=OP.mult,
                        )
                    h_prev, c_prev = h_new, c_new
                    if s + 1 < S:
                        pbk = nxt

              # ------------- epilogue -------------
              with tc.tile_pool(name="psum2", bufs=1, space="PSUM") as psum2:
                mh = work.tile([128, KC * BL], F16, tag="mh", name="mh")
                nc.vector.tensor_mul(mh[:, :], h_prev[:, :], mot[:, :])
                pso = psum2.tile([1, BL], F32, tag="pso", name="pso", bufs=1)
                for k in range(KC):
                    nc.tensor.matmul(
                        pso[0:1, :],
                        wl16[:, k:k + 1],
                        mh[:, BL * k:BL * (k + 1)],
                        start=(k == 0), stop=(k == KC - 1),
                    )
                osb = work.tile([1, BL], F32, tag="osb", name="osb")
                nc.vector.tensor_scalar(
                    osb[:, :], pso[0:1, :], bl32[0:1, 0:1], None, op0=OP.add
                )
                nc.sync.dma_start(out_d.rearrange("b o -> o b"), osb[:, :])

    nc.compile()
    _cache["built"] = nc
    return nc


def kernel(**inputs) -> np.ndarray:
    nc = _build()
    f32 = np.float32
    f16 = np.float16

    def c(a):
        return np.asarray(a, dtype=f32)

    # ---- host-side preprocessing (variational sampling + layout) ----
    sp_ih = np.logaddexp(0.0, c(inputs["W_ih_rho"]))       # softplus
    W_ih = (c(inputs["W_ih_mu"]) + sp_ih * c(inputs["eps_ih"])).reshape(G4)
    sp_b = np.logaddexp(0.0, c(inputs["b_rho"]))
    bias = (c(inputs["b_mu"]) + sp_b * c(inputs["eps_b"])).reshape(G4)
    sp_hh = np.logaddexp(0.0, c(inputs["W_hh_rho"]))
    W_hh = c(inputs["W_hh_mu"]) + sp_hh * c(inputs["eps_hh"])   # (H, 4H)

    # scaling tricks: tanh(x) = 2*sig(2x)-1 needs the g-gate (cols
    # 1024:1536) pre-activations doubled; storing h/2 needs all W_hh
    # columns doubled (and W_lin doubled at the output).
    sc_row = np.ones(G4, f32)
    sc_row[1024:1536] = 2.0
    sc_w = np.full(G4, 2.0, f32)
    sc_w[1024:1536] = 4.0
    wg_np = np.ascontiguousarray(
        np.stack([W_ih * sc_row, bias * sc_row]).astype(f16)
    )
    whh16 = (W_hh * sc_w).astype(f16)                       # (512, 2048)
    w16_np = [
        np.ascontiguousarray(whh16[128 * k:128 * (k + 1)]) for k in range(KC)
    ]
    wl2 = (2.0 * c(inputs["W_lin"]).reshape(KC, 128).T).astype(f16)
    wl16_np = np.ascontiguousarray(wl2)                     # (128, KC)
    bl_np = c(inputs["b_lin"]).reshape(1, OUT)

    u_full = (c(inputs["x"]) * c(inputs["mask_in"]).reshape(B, S)).astype(f16)
    mo_full = c(inputs["mask_out"]).astype(f16)             # (B, H)

    w16cat_np = np.ascontiguousarray(np.concatenate(w16_np, axis=1))
    shared = {"wg_in": wg_np, "w16_in": w16cat_np, "bl_in": bl_np}
    in_maps = []
    for i in range(N_CORES):
        sl = slice(BL * i, BL * (i + 1))
        m = dict(shared)
        u_sl = u_full[sl]                                   # (BL, S)
        m["u2_in"] = np.ascontiguousarray(
            np.stack([u_sl.T.ravel(), np.ones(S * BL, f16)])
        )
        mo_sl = mo_full[sl]                                 # (BL, H)
        m["mw_in"] = np.ascontiguousarray(
            np.concatenate(
                [mo_sl[:, 128 * k:128 * (k + 1)].T for k in range(KC)]
                + [wl16_np], axis=1
            )
        )
        in_maps.append(m)

    trace = bool(int(os.environ.get("KERNEL_TRACE", "0")))
    trace_cores = None
    if trace and int(os.environ.get("KERNEL_TRACE_ALL", "0")):
        trace_cores = list(range(N_CORES))
    res = None
    last_err = None
    for _attempt in range(3):
        try:
            res = run_bass_kernel_spmd(
                nc, in_maps, core_ids=list(range(N_CORES)), trace=trace,
                trace_cores=trace_cores,
            )
            break
        except Exception as e:  # transient NRT/device hiccups: retry
            last_err = e
    if res is None:
        raise last_err
    _cache["last_results"] = res
    out = np.concatenate(
        [res.results[i]["out"].reshape(BL, OUT) for i in range(N_CORES)], axis=0
    )
    return out.astype(np.float32)
